# revision 25
# baseline (speedup 1.0000x reference)
"""Causal self-attention Bass kernel for 8 TRN2 NeuronCores.

Problem: B=4, T=2048, C=1024, H=16 heads, head_dim=64, fp32.
    q = x @ Wq.T ; k = x @ Wk.T ; v = x @ Wv.T          (per head)
    att = softmax(mask(q k^T / 8))
    y = att @ v ; out = y @ Wp.T

Sharding (8 cores): 4-way data parallel over batch x 2-way tensor
parallel over heads. Core c handles batch c//2 and heads 8*(c%2)..+8.
Wq/Wk/Wv column-parallel, Wp row-parallel; the partial outputs of the
two head-halves of each batch are summed on the host (the "all-reduce"
of row-parallel Wp).

Everything on-device is bf16 (host pre-casts inputs, halves DMA and
enables the PE fast-weight-load path); PSUM accumulation is fp32, so
end-to-end l2 rel err stays ~5e-3 against the 2e-2 gate.

Single fused pipeline: per 512-token q-chunk qc, the attention kt-loop
(scores -> exp -> PV) is exp(ACT)-paced, which leaves PE bubbles; the
q/k/v projection matmuls of chunk qc+1 (and, during the last chunk,
all the output-projection matmuls) are injected as filler ops inside
the loop so the TensorEngine never idles. Scores matmuls of the two
head-halves run concurrently in 64x128 row-tiled PE mode (K=64 each),
and the moving operand is restricted to the causal columns on
diagonal blocks. The output projection keeps Wp stationary and yT
moving, producing outT [C, T]; the host transposes. The last chunk's
outproj is split ji 0..2 / ji 3 so only one matmul per c-block trails
the final attention segment, whose softmax normalization broadcasts
the reciprocal through a PE ones-matmul instead of the DRAM bounce.

Measured on 8 trn2 cores: ~275-280us HW exec (baseline 346us), l2
rel err 4.6e-3. TensorMatrix is ~86% busy; op durations inflate ~15%
when many engines run concurrently (power/arbitration), which is why
further overlap stopped paying. Run-to-run variance is +/-20% (the
first run after a fresh compile is often a ~330us outlier), and the
device occasionally corrupts a run's numerics (seen once at 1.9e-2,
once at 1.4e-1 on the unmodified baseline kernel) -- rerun before
trusting a single bad measurement.
"""

from contextlib import ExitStack

import numpy as np

import concourse.bass as bass
import concourse.tile as tile
from concourse import bacc, mybir

F32 = mybir.dt.float32
BF16 = mybir.dt.bfloat16

B, T, C, H, D = 4, 2048, 1024, 16, 64
NCORES = 8
JL = 512            # local j dims per core (8 heads * 64)
NPAIR = 4           # local head pairs
CI = C // 128       # 8 c-tiles
NT = T // 128       # 16 t/k tiles
NQC = T // 512      # 4 q chunks
VW = D + 1          # v columns per head incl. the ones column

_CACHED_NC = None


def build_nc():
    nc = bacc.Bacc(None)

    xT = nc.dram_tensor("xT", [C, T], BF16, kind="ExternalInput")
    wqT = nc.dram_tensor("wqT", [C, JL], BF16, kind="ExternalInput")
    wkT = nc.dram_tensor("wkT", [C, JL], BF16, kind="ExternalInput")
    wvT = nc.dram_tensor("wvT", [C, JL], BF16, kind="ExternalInput")
    wpT = nc.dram_tensor("wpT", [JL, C], BF16, kind="ExternalInput")
    outT = nc.dram_tensor("outT", [C, T], BF16, kind="ExternalOutput")
    # bounce buffer for broadcasting softmax reciprocals across partitions
    rcd = nc.dram_tensor("rcd", [NPAIR, NQC, 2, 512], F32)

    xT_r = xT.rearrange("(ci p) t -> p ci t", p=128)
    wq_r = wqT.rearrange("(ci p) j -> p ci j", p=128)
    wk_r = wkT.rearrange("(ci p) j -> p ci j", p=128)
    wv_r = wvT.rearrange("(ci p) j -> p ci j", p=128)
    wp_r = wpT.rearrange("(ji p) c -> p ji c", p=128)
    outT_r = outT.rearrange("(cb p) t -> p cb t", p=128)

    with tile.TileContext(nc) as tc, ExitStack() as ctx:
        pm = ctx.enter_context(tc.tile_pool(name="pm", bufs=1))
        expp = ctx.enter_context(tc.tile_pool(name="expp", bufs=6))
        bcp = ctx.enter_context(tc.tile_pool(name="bcp", bufs=2))
        rcp = ctx.enter_context(tc.tile_pool(name="rcp", bufs=2))
        ycp = ctx.enter_context(tc.tile_pool(name="ycp", bufs=2))
        stp = ctx.enter_context(tc.tile_pool(name="stp", bufs=2))
        outp = ctx.enter_context(tc.tile_pool(name="outp", bufs=3))
        # PSUM budget (8 banks): scores 2x2 + y 2x1 + filler acc 2x1
        gp = ctx.enter_context(tc.tile_pool(name="gp", bufs=2, space="PSUM"))
        yp = ctx.enter_context(tc.tile_pool(name="yp", bufs=2, space="PSUM"))
        accp = ctx.enter_context(tc.tile_pool(name="accp", bufs=2, space="PSUM"))

        # persistent sbuf tensors
        xt_sb = pm.tile([128, CI, T], BF16, tag="xt")
        wq_sb = pm.tile([128, CI, JL], BF16, tag="wq")
        wk_sb = pm.tile([128, CI, JL], BF16, tag="wk")
        wv_sb = pm.tile([128, CI, JL], BF16, tag="wv")
        wp_sb = pm.tile([128, NPAIR, C], BF16, tag="wp")
        qT_all = pm.tile([128, NPAIR, T], BF16, tag="qT")
        kT_all = pm.tile([128, NPAIR, T], BF16, tag="kT")
        yT_all = pm.tile([128, NPAIR, T], BF16, tag="yT")
        # v with a ones column prepended per head (so the softmax sums land
        # on psum partition 0), plus 64 pad columns so every per-head lhsT
        # can be read as [128, 128] (NumWeights==128 -> fast weight load).
        v_sb = pm.tile([128, NT, 8 * VW + 64], BF16, tag="v")
        v_view = v_sb[:, :, 0 : 8 * VW].rearrange("p n (h w) -> p n h w", w=VW)
        ones_col = pm.tile([128, NT, 8, 1], F32, tag="ones")
        ones_row_u = pm.tile([1, D + 1], BF16, tag="ones_row")
        # ji 0..2 partial sums of the last chunk's output projection
        part_sb = pm.tile([128, 8, 512], BF16, tag="part")

        # ---- input DMAs ------------------------------------------------
        # wq/x-chunk-0 interleaved per c-tile so the first projection
        # matmuls start ~2us in, instead of waiting out the sync queue's
        # serial ~650ns dispatches; the rest batched coarse
        for ci in range(CI):
            nc.sync.dma_start(wq_sb[:, ci, :], wq_r[:, ci, :])
            nc.sync.dma_start(xt_sb[:, ci, 0:512], xT_r[:, ci, 0:512])
        nc.sync.dma_start(wk_sb[:], wk_r[:])
        nc.sync.dma_start(wv_sb[:], wv_r[:])
        for tch in range(1, NQC):
            ts_ = slice(tch * 512, tch * 512 + 512)
            nc.sync.dma_start(xt_sb[:, :, ts_], xT_r[:, :, ts_])
        nc.sync.dma_start(wp_sb[:], wp_r[:])

        # warm the PE clock gate (HAM un-throttles after ~3.4us of
        # activity) with junk matmuls while the first input DMAs land;
        # ones_row is memset afterwards and warm is never read
        warm_src = pm.tile([1, 512], BF16, tag="warmsrc")
        nc.vector.memset(warm_src[:], 1.0)
        nc.vector.memset(ones_row_u[:], 1.0)
        warm = accp.tile([128, 512], F32, tag="acc", name="warm")
        for _ in range(12):
            nc.tensor.matmul(
                warm[0 : D + 1, :], ones_row_u[0:1, 0 : D + 1],
                warm_src[0:1, :], start=True, stop=True,
            )

        nc.vector.memset(ones_col[:], 1.0)
        nc.vector.tensor_copy(v_view[:, :, :, 0:1], ones_col[:])
        nc.vector.memset(v_sb[:, :, 8 * VW : 8 * VW + 64], 0.0)
        ones_row = ones_row_u

        # ---- emitters ---------------------------------------------------
        def mk_acc(pool):
            # the gp pool's buffers are 2-bank "g" tiles; reuse them for
            # prologue/epilogue accumulators so no extra PSUM is reserved
            if pool is gp:
                t = pool.tile([128, 2, 512], F32, tag="g", name="gacc")
                return t[:, 0, :]
            return pool.tile([128, 512], F32, tag="acc", name="acc")

        def proj_gen(tch, pool):
            """q/k/v projections for t-chunk tch; yields after each inst."""
            ts_ = slice(tch * 512, tch * 512 + 512)
            for w_sb, dst in ((wq_sb, qT_all), (wk_sb, kT_all)):
                for pr in range(NPAIR):
                    acc = mk_acc(pool)
                    for ci in range(CI):
                        nc.tensor.matmul(
                            acc[:],
                            w_sb[:, ci, pr * 128 : pr * 128 + 128],
                            xt_sb[:, ci, ts_],
                            start=(ci == 0),
                            stop=(ci == CI - 1),
                        )
                        yield
                    nc.vector.tensor_copy(dst[:, pr, ts_], acc[:])
                    yield
            for tl in range(4):
                ti = tch * 4 + tl
                acc = mk_acc(pool)
                for ci in range(CI):
                    nc.tensor.matmul(
                        acc[:],
                        xt_sb[:, ci, tch * 512 + tl * 128 : tch * 512 + tl * 128 + 128],
                        wv_sb[:, ci, :],
                        start=(ci == 0),
                        stop=(ci == CI - 1),
                    )
                    yield
                nc.vector.tensor_copy(
                    v_view[:, ti, :, 1 : D + 1],
                    acc[:].rearrange("p (h d) -> p h d", d=D),
                )
                yield

        def outproj_gen(tchs, pool):
            """output projection outT[c,t] (Wp stationary) for contiguous
            t-chunks tchs, one batched DMA per 128-row c-block."""
            lo = tchs[0] * 512
            for cb in range(8):
                o = outp.tile([128, len(tchs), 512], BF16, tag="o", name="o")
                for idx, tch in enumerate(tchs):
                    ts_ = slice(tch * 512, tch * 512 + 512)
                    acc = mk_acc(pool)
                    for ji in range(NPAIR):
                        nc.tensor.matmul(
                            acc[:],
                            wp_sb[:, ji, cb * 128 : cb * 128 + 128],
                            yT_all[:, ji, ts_],
                            start=(ji == 0),
                            stop=(ji == NPAIR - 1),
                        )
                        yield
                    nc.vector.tensor_copy(o[:, idx, :], acc[:])
                    yield
                nc.sync.dma_start(outT_r[:, cb, lo : lo + len(tchs) * 512], o[:])
                yield

        def outproj3_partial_gen(pool):
            """ji 0..2 partial accumulation of outproj(last chunk), runnable
            as soon as pr 0..2 of the last chunk are normalized."""
            ts_ = slice((NQC - 1) * 512, NQC * 512)
            for cb in range(8):
                acc = mk_acc(pool)
                for ji in range(NPAIR - 1):
                    nc.tensor.matmul(
                        acc[:],
                        wp_sb[:, ji, cb * 128 : cb * 128 + 128],
                        yT_all[:, ji, ts_],
                        start=(ji == 0),
                        stop=(ji == NPAIR - 2),
                    )
                    yield
                nc.vector.tensor_copy(part_sb[:, cb, :], acc[:])
                yield

        class Fillers:
            def __init__(self):
                self.gens = []
                self.n = 0

            def add(self, gen, n):
                self.gens.append(gen)
                self.n += n

            def run(self, k):
                while k > 0 and self.gens:
                    try:
                        next(self.gens[0])
                        self.n -= 1
                        k -= 1
                    except StopIteration:
                        self.gens.pop(0)

            def drain(self):
                self.run(1 << 30)

        N_PROJ = 2 * NPAIR * (CI + 1) + 4 * (CI + 1)     # 108 items

        # ---- chunk-0 projections (before any attention) -----------------
        for _ in proj_gen(0, gp):
            pass

        # ---- fused attention + fillers ----------------------------------
        # proj(qc+1) must run during qc; outproj is deferred to qc=3 where
        # the exp-paced loop has the most spare PE time
        for qc in range(NQC):
            fill = Fillers()
            if qc < NQC - 1:
                fill.add(proj_gen(qc + 1, accp), N_PROJ)
            else:
                fill.add(
                    outproj_gen(list(range(NQC - 1)), accp),
                    8 * ((NQC - 1) * (NPAIR + 1) + 1),
                )
            iters_left = NPAIR * (4 * qc + 4)

            for pr in range(NPAIR):
                if qc == NQC - 1 and pr == NPAIR - 1:
                    fill.add(outproj3_partial_gen(accp), 8 * NPAIR)
                qlo = qT_all[0:64, pr, :]
                qhi = qT_all[64:128, pr, :]
                klo = kT_all[0:64, pr, :]
                khi = kT_all[64:128, pr, :]
                nkt = 4 * qc + 4
                q0 = qc * 512
                yA = yp.tile([128, 512], F32, tag="y")
                yB = yp.tile([128, 512], F32, tag="y")

                # software pipeline: issue scores/exp for kt before the PV
                # matmuls of kt-1, so the PE never waits on ACT's exp.
                # lhsT is [128, 128] (head's ones+v then pad/next-head
                # cols); psum rows 65..127 are don't-care junk.
                def emit_pv(kt, e, nkt=nkt, yA=yA, yB=yB, pr=pr, qc=qc):
                    dt = kt - 4 * qc
                    lo = dt * 128 if dt > 0 else 0
                    nc.tensor.matmul(
                        yA[:, lo:512],
                        v_sb[:, kt, 2 * pr * VW : 2 * pr * VW + 128],
                        e[:, 0, lo:512],
                        start=(kt == 0),
                        stop=(kt == nkt - 1),
                    )
                    nc.tensor.matmul(
                        yB[:, lo:512],
                        v_sb[:, kt, (2 * pr + 1) * VW : (2 * pr + 1) * VW + 128],
                        e[:, 1, lo:512],
                        start=(kt == 0),
                        stop=(kt == nkt - 1),
                    )

                prev = None
                for kt in range(nkt):
                    dt = kt - 4 * qc
                    xlo = dt * 128 if dt > 0 else 0
                    ks = slice(kt * 128, kt * 128 + 128)
                    qs = slice(q0 + xlo, q0 + 512)
                    g = gp.tile([128, 2, 512], F32, tag="g")
                    nc.tensor.matmul(
                        g[:, 0, xlo:512], klo[:, ks], qlo[:, qs],
                        start=True, stop=True,
                    )
                    nc.tensor.matmul(
                        g[:, 1, xlo:512], khi[:, ks], qhi[:, qs],
                        start=True, stop=True,
                    )
                    e = expp.tile([128, 2, 512], BF16, tag="e")
                    nc.scalar.activation(
                        e[:, :, xlo:512],
                        g[:, :, xlo:512],
                        mybir.ActivationFunctionType.Exp,
                        scale=0.125,
                    )
                    if dt >= 0:
                        # zero the causal triangle (k > q) of the diagonal
                        # block, on the otherwise-idle gpsimd engine
                        bs = slice(dt * 128, dt * 128 + 128)
                        for h in (0, 1):
                            nc.gpsimd.affine_select(
                                out=e[:, h, bs],
                                in_=e[:, h, bs],
                                compare_op=mybir.AluOpType.is_ge,
                                fill=0.0,
                                base=0,
                                pattern=[[1, 128]],
                                channel_multiplier=-1,
                            )
                    # fillers between the scores and the dependent PV ops
                    # keep the PE busy while ACT computes the exp
                    rate = -(-fill.n // iters_left) if iters_left > 0 else fill.n
                    fill.run(rate)
                    iters_left -= 1
                    if prev is not None:
                        emit_pv(*prev)
                    prev = (kt, e)
                # cover the pipeline drain: the final PV waits on the last
                # exp, so give the PE a few fillers first
                fill.run(3)
                emit_pv(*prev)

                # evacuate y psum -> sbuf immediately (frees psum banks)
                yc = ycp.tile([D + 1, 2, 512], F32, tag="yc")
                nc.vector.tensor_copy(yc[0 : D + 1, 0, :], yA[0 : D + 1, :])
                nc.vector.tensor_copy(yc[0 : D + 1, 1, :], yB[0 : D + 1, :])
                # normalize: y / rowsum (sums live in row 0 = partition 0)
                rc = rcp.tile([1, 2, 512], F32, tag="rc")
                nc.vector.reciprocal_approx_fast(rc[0:1, :, :], yc[0:1, :, :])
                last_seg = qc == NQC - 1 and pr == NPAIR - 1
                if last_seg:
                    # broadcast via the PE (ones-column matmul into the freed
                    # y banks) -- the DRAM bounce's ~4us of DMA-chain latency
                    # would sit exposed on the critical path here
                    bcA = yp.tile([128, 512], F32, tag="y", name="bcA")
                    bcB = yp.tile([128, 512], F32, tag="y", name="bcB")
                    rcb = rcp.tile([1, 2, 512], BF16, tag="rcb", name="rcb")
                    nc.vector.tensor_copy(rcb[:], rc[:])
                    nc.tensor.matmul(
                        bcA[0 : D + 1, :], ones_row[0:1, 0 : D + 1],
                        rcb[0:1, 0, :], start=True, stop=True,
                    )
                    nc.tensor.matmul(
                        bcB[0 : D + 1, :], ones_row[0:1, 0 : D + 1],
                        rcb[0:1, 1, :], start=True, stop=True,
                    )
                    # mul reads the broadcast straight from psum
                    stg = stp.tile([D + 1, 2, 512], BF16, tag="stg")
                    nc.vector.tensor_mul(
                        stg[0 : D + 1, 0, :], yc[0 : D + 1, 0, :], bcA[0 : D + 1, :]
                    )
                    nc.vector.tensor_mul(
                        stg[0 : D + 1, 1, :], yc[0 : D + 1, 1, :], bcB[0 : D + 1, :]
                    )
                else:
                    bc = bcp.tile([D + 1, 2, 512], F32, tag="bc")
                    nc.sync.dma_start(rcd[pr, qc, :, :], rc[0:1, :, :])
                    s = rcd[pr, qc, :, :]
                    src = bass.AP(
                        tensor=s.tensor,
                        offset=s.offset,
                        ap=[[0, D + 1]] + list(s.ap),
                    )
                    nc.sync.dma_start(bc[0 : D + 1, :, :], src)
                    # row 0 is the sums row scaled by its own reciprocal --
                    # discarded; rows 1..64 repartition into yT_all via DMA
                    stg = stp.tile([D + 1, 2, 512], BF16, tag="stg")
                    nc.vector.tensor_mul(
                        stg[0 : D + 1, :, :], yc[0 : D + 1, :, :], bc[0 : D + 1, :, :]
                    )
                qs_full = slice(q0, q0 + 512)
                nc.sync.dma_start(yT_all[0:64, pr, qs_full], stg[1 : D + 1, 0, :])
                nc.sync.dma_start(yT_all[64:128, pr, qs_full], stg[1 : D + 1, 1, :])

            fill.drain()

        # ---- epilogue: last chunk's outproj, ji=3 only (0..2 were
        # accumulated into part_sb during the last attention segment) ------
        ts3 = slice((NQC - 1) * 512, NQC * 512)
        o3 = outp.tile([128, 8, 512], BF16, tag="o3", bufs=1)
        for cb in range(8):
            acc = mk_acc(gp)
            nc.tensor.matmul(
                acc[:],
                wp_sb[:, NPAIR - 1, cb * 128 : cb * 128 + 128],
                yT_all[:, NPAIR - 1, ts3],
                start=True,
                stop=True,
            )
            nc.vector.tensor_add(o3[:, cb, :], acc[:], part_sb[:, cb, :])
            if cb % 2 == 1:
                # stagger the writeback so the final transfer isn't one
                # exposed 1MB DMA after all compute is done
                nc.sync.dma_start(
                    outT_r[:, cb - 1 : cb + 1, ts3], o3[:, cb - 1 : cb + 1, :]
                )

    nc.finalize()
    return nc


def _get_nc():
    global _CACHED_NC
    if _CACHED_NC is None:
        _CACHED_NC = build_nc()
    return _CACHED_NC


def kernel(x, Wq, Wk, Wv, Wp):
    import ml_dtypes
    from concourse.bass_utils import run_bass_kernel_spmd

    bf16 = ml_dtypes.bfloat16
    x = np.asarray(x, dtype=np.float32)
    Wq = np.asarray(Wq, dtype=np.float32)
    Wk = np.asarray(Wk, dtype=np.float32)
    Wv = np.asarray(Wv, dtype=np.float32)
    Wp = np.asarray(Wp, dtype=np.float32)

    nc = _get_nc()

    xT = [np.ascontiguousarray(x[b].T).astype(bf16) for b in range(B)]
    wqT, wkT, wvT, wpT = [], [], [], []
    for hh in range(2):
        js = slice(JL * hh, JL * hh + JL)
        wqT.append(np.ascontiguousarray(Wq[js, :].T).astype(bf16))
        wkT.append(np.ascontiguousarray(Wk[js, :].T).astype(bf16))
        wvT.append(np.ascontiguousarray(Wv[js, :].T).astype(bf16))
        wpT.append(np.ascontiguousarray(Wp[:, js].T).astype(bf16))

    in_maps = []
    for c in range(NCORES):
        b, hh = c // 2, c % 2
        in_maps.append(
            {
                "xT": xT[b],
                "wqT": wqT[hh],
                "wkT": wkT[hh],
                "wvT": wvT[hh],
                "wpT": wpT[hh],
            }
        )

    res = run_bass_kernel_spmd(nc, in_maps, core_ids=list(range(NCORES)))

    out = np.empty((B, T, C), dtype=np.float32)
    for b in range(B):
        partial = res.results[2 * b]["outT"].astype(np.float32) + res.results[
            2 * b + 1
        ]["outT"].astype(np.float32)
        out[b] = partial.T
    return out
